# revision 12
# baseline (speedup 1.0000x reference)
"""Multi-head attention (BaseMultiHeadAttention) Trainium2 Bass kernel.

Problem: m=4, nq=nkv=2048, qk_dim=v_dim=1024, 16 heads x 64 head_dim,
fp32, out = softmax(Q K^T / 8) V projected by Wo + bo.

Sharding over 8 cores: core c = (batch b=c//2, head-group g=c%2).
Each core computes 8 heads of one batch:
  - Q^T, K^T projections in [d, n] layout (weights stationary; host
    supplies x^T so no on-device transposes are needed)
  - V in natural [n, d] layout, augmented with a ones column per head so
    the attention-weight matmul also produces the softmax denominator
  - S^T = K Q^T per (head, q-chunk of 512); exp via ScalarE (scale fused)
  - O^T = V^T-free accumulation: lhsT = V chunk, rhs = exp(S^T) chunk
  - normalize by broadcast-matmul of 1/denominator, then row-parallel
    out-projection with Wo^T; partial outputs summed on host (+ bias).

All matmuls run operands bitcast to float32r (tensor engine processes
fp32 data 12-bit-mantissa-rounded at full rate for free dims >= 256).
"""

import numpy as np

import concourse.bass as bass
from concourse import bacc
import concourse.mybir as mybir
import concourse.tile as tile
from concourse.bass_utils import run_bass_kernel_spmd

F32 = mybir.dt.float32
F32R = mybir.dt.float32r
P = 128


# --------------------------------------------------------------------------
# Workaround: the walrus build in this container rejects Drain instructions
# carrying more than one sync wait (CoreV3GenImpl setupSyncWait). Split the
# TileContext tail drain into a chain of drains, one wait each.
def _install_drain_patch():
    import concourse.tile as _tile
    import concourse.mybir as _mybir
    from concourse.vector_clock import ScopedClock as _ScopedClock

    if getattr(_tile.TileContext, "_drain_split_patch", False):
        return

    def _patched_drain_and_barrier(self, tick_clock, wait_clock):
        drain_inst = self.nc.sync.drain()
        wait_clock.add_sem_waits(
            drain_inst.ins, _ScopedClock({None: tick_clock.global_clock})
        )
        si = drain_inst.ins.sync_info
        if si is not None and len(si.on_wait) > 1:
            waits = list(si.on_wait)
            drain_inst.ins.sync_info = _mybir.SyncInfo(
                on_wait=[waits[0]], on_update=list(si.on_update)
            )
            for w in waits[1:]:
                extra = self.nc.sync.drain()
                extra.ins.sync_info = _mybir.SyncInfo(on_wait=[w], on_update=[])
        self.nc.all_engine_barrier()
        assert self.sems is not None
        popped = self.nc._tile_sem_poison_stack.pop()
        assert popped is self._sem_poison
        self.nc.clear_and_free_semaphores(list(self.sems.allocated().values()))
        self.nc.all_engine_barrier()

    _tile.TileContext._drain_and_barrier = _patched_drain_and_barrier
    _tile.TileContext._drain_split_patch = True


_install_drain_patch()


def build_core_program(
    nq=2048,
    nkv=2048,
    ckdim=1024,  # qk/v input feature dim
    n_heads=8,  # heads handled by this core
    hd=64,  # head dim
    odim=1024,  # output dim of Wo
    cs=512,  # free-dim chunk size for matmuls
    num_devices=8,
):
    """Emit the per-core SPMD program. Inputs (per core):
    xqT/xkT/xvT [ckdim, nq|nkv], wqT/wkT/wvT [ckdim, d_local],
    woT [d_local, odim]. Output: outT [odim, nq] (partial, pre-bias).
    """
    d_local = n_heads * hd
    scale = float(hd) ** -0.5
    KC = ckdim // P  # contraction chunks for projections
    DMT = d_local // P  # partition tiles of Q^T/K^T
    NQC = nq // cs  # q chunks
    NVC = nkv // cs  # kv chunks (for V projection)
    NKT = nkv // P  # kv partition tiles (attention contraction)
    OT = odim // P  # out-proj partition tiles
    SUB = cs // P  # 128-row subtiles per chunk
    HPT = P // hd  # heads per partition tile
    assert d_local % P == 0 and nq % cs == 0 and nkv % cs == 0
    assert odim % P == 0 and ckdim % P == 0 and cs % P == 0

    nc = bacc.Bacc(
        "TRN2", target_bir_lowering=False, debug=False, num_devices=num_devices
    )
    xqT = nc.declare_dram_parameter("xqT", [ckdim, nq], F32R, isOutput=False)
    xkT = nc.declare_dram_parameter("xkT", [ckdim, nkv], F32R, isOutput=False)
    xvT = nc.declare_dram_parameter("xvT", [ckdim, nkv], F32R, isOutput=False)
    wqT = nc.declare_dram_parameter("wqT", [ckdim, d_local], F32R, isOutput=False)
    wkT = nc.declare_dram_parameter("wkT", [ckdim, d_local], F32R, isOutput=False)
    wvT = nc.declare_dram_parameter("wvT", [ckdim, d_local], F32R, isOutput=False)
    woT = nc.declare_dram_parameter("woT", [d_local, odim], F32R, isOutput=False)
    outT = nc.declare_dram_parameter("outT", [odim, nq], F32, isOutput=True)

    with tile.TileContext(nc) as tc:
        import contextlib

        ctx = contextlib.ExitStack()
        with ctx:
            ctx.enter_context(
                nc.allow_low_precision(
                    reason="fp32r is fp32 with a 12-bit-rounded mantissa; "
                    "accumulation stays fp32 in PSUM"
                )
            )
            w_pool = ctx.enter_context(tc.tile_pool(name="w", bufs=1))
            wo_pool = ctx.enter_context(tc.tile_pool(name="wo", bufs=1))
            qt_pool = ctx.enter_context(tc.tile_pool(name="qt", bufs=1))
            kt_pool = ctx.enter_context(tc.tile_pool(name="kt", bufs=1))
            v_pool = ctx.enter_context(tc.tile_pool(name="v", bufs=1))
            xin_pool = ctx.enter_context(tc.tile_pool(name="xin", bufs=6))
            exp_pool = ctx.enter_context(tc.tile_pool(name="expp", bufs=4))
            ot_pool = ctx.enter_context(tc.tile_pool(name="ot", bufs=2))
            rec_pool = ctx.enter_context(tc.tile_pool(name="rec", bufs=2))
            out_pool = ctx.enter_context(tc.tile_pool(name="outp", bufs=3))
            bcs_pool = ctx.enter_context(tc.tile_pool(name="bcs", bufs=2))
            ones_pool = ctx.enter_context(tc.tile_pool(name="ones", bufs=1))
            ps_mm = ctx.enter_context(tc.tile_pool(name="ps_mm", bufs=4, space="PSUM"))
            ps_st = ctx.enter_context(tc.tile_pool(name="ps_st", bufs=2, space="PSUM"))
            ps_ot = ctx.enter_context(tc.tile_pool(name="ps_ot", bufs=1, space="PSUM"))
            ps_bc = ctx.enter_context(tc.tile_pool(name="ps_bc", bufs=1, space="PSUM"))

            # ---- resident weights --------------------------------------
            def load_w(dram, label):
                tiles = []
                for kc in range(KC):
                    t = w_pool.tile([P, d_local], F32R, name=f"{label}{kc}", tag=f"w{kc}")
                    nc.sync.dma_start(out=t, in_=dram[kc * P : (kc + 1) * P, :])
                    tiles.append(t)
                return tiles

            wq_sb = load_w(wqT, "wq")
            wk_sb = load_w(wkT, "wk")

            wo_sb = []
            for mt in range(DMT):
                t = wo_pool.tile([P, odim], F32R, name=f"wo{mt}")
                nc.sync.dma_start(out=t, in_=woT[mt * P : (mt + 1) * P, :])
                wo_sb.append(t)

            # ones row at partition hd*? -> used as lhsT for the
            # denominator broadcast matmul (base partition = hd)
            ones_t = ones_pool.tile([P, hd], F32R, name="ones_t")
            nc.vector.memset(ones_t.bitcast(F32), 1.0)

            # ---- projections -------------------------------------------
            # Q^T and K^T: [d_local, n] tiles; weights stationary per kc.
            def project_T(x_dram, w_sb, out_pool, label, n_tot):
                out_tiles = [
                    out_pool.tile([P, n_tot], F32R, name=f"{label}{mt}")
                    for mt in range(DMT)
                ]
                for nch in range(n_tot // cs):
                    xch = []
                    for kc in range(KC):
                        t = xin_pool.tile([P, cs], F32R, name=f"x_{label}_{nch}_{kc}",
                                          tag="xin")
                        nc.sync.dma_start(
                            out=t,
                            in_=x_dram[
                                kc * P : (kc + 1) * P, nch * cs : (nch + 1) * cs
                            ],
                        )
                        xch.append(t)
                    pss = [
                        ps_mm.tile([P, cs], F32, name=f"ps_{label}_{nch}_{mt}",
                                   tag="psmm")
                        for mt in range(DMT)
                    ]
                    for kc in range(KC):
                        for mt in range(DMT):
                            nc.tensor.matmul(
                                pss[mt],
                                lhsT=w_sb[kc][:, mt * P : (mt + 1) * P],
                                rhs=xch[kc],
                                start=(kc == 0),
                                stop=(kc == KC - 1),
                            )
                    for mt in range(DMT):
                        nc.vector.tensor_copy(
                            out=out_tiles[mt][:, nch * cs : (nch + 1) * cs],
                            in_=pss[mt],
                        )
                return out_tiles

            qt_sb = project_T(xqT, wq_sb, qt_pool, "qt", nq)
            kt_sb = project_T(xkT, wk_sb, kt_pool, "kt", nkv)

            # V natural layout, heads interleaved with a ones column:
            # v_t[nt] is [128, n_heads, hd+1]; [:, h, 0:hd] = V rows for
            # head h, [:, h, hd] = 1.0
            wv_sb = load_w(wvT, "wv")
            v_t = [
                v_pool.tile([P, n_heads, hd + 1], F32R, name=f"v{nt}")
                for nt in range(NKT)
            ]
            for nt in range(NKT):
                nc.vector.memset(v_t[nt][:, :, hd : hd + 1].bitcast(F32), 1.0)
            for nch in range(NVC):
                xch = []
                for kc in range(KC):
                    t = xin_pool.tile([P, cs], F32R, name=f"x_v_{nch}_{kc}", tag="xin")
                    nc.sync.dma_start(
                        out=t,
                        in_=xvT[kc * P : (kc + 1) * P, nch * cs : (nch + 1) * cs],
                    )
                    xch.append(t)
                pss = [
                    ps_mm.tile([P, d_local], F32, name=f"ps_v_{nch}_{sub}", tag="psmm")
                    for sub in range(SUB)
                ]
                for kc in range(KC):
                    for sub in range(SUB):
                        nc.tensor.matmul(
                            pss[sub],
                            lhsT=xch[kc][:, sub * P : (sub + 1) * P],
                            rhs=wv_sb[kc],
                            start=(kc == 0),
                            stop=(kc == KC - 1),
                        )
                for sub in range(SUB):
                    nt = nch * SUB + sub
                    nc.vector.tensor_copy(
                        out=v_t[nt][:, :, 0:hd],
                        in_=pss[sub].rearrange("p (h d) -> p h d", h=n_heads),
                    )

            # ---- attention + out-projection per q-chunk ----------------
            # ot_t stacks head pairs on partitions: [:, hp, :] holds heads
            # (2hp, 2hp+1) at partitions [0:hd) and [hd:2hd). The odd-head
            # normalize is a partition-shifted DVE write (64-channel ops may
            # write quadrants 2/3 from src 0/1).
            n_hp = n_heads // HPT
            for qg in range(NQC):
                qsl = slice(qg * cs, (qg + 1) * cs)
                ot_t = ot_pool.tile([P, n_hp, cs], F32R, name=f"ot{qg}", tag="ot")
                for h in range(n_heads):
                    mt = h // HPT
                    poff = (h % HPT) * hd
                    ot_ps = ps_ot.tile([hd + 1, cs], F32, name=f"otps{qg}_{h}",
                                       tag="otps")
                    for kt in range(NKT):
                        st = ps_st.tile([P, cs], F32, name=f"st{qg}_{h}_{kt}",
                                        tag="st")
                        nc.tensor.matmul(
                            st,
                            lhsT=kt_sb[mt][
                                poff : poff + hd, kt * P : (kt + 1) * P
                            ],
                            rhs=qt_sb[mt][poff : poff + hd, qsl],
                            start=True,
                            stop=True,
                        )
                        ex = exp_pool.tile([P, cs], F32R, name=f"ex{qg}_{h}_{kt}",
                                           tag="ex")
                        nc.scalar.activation(
                            out=ex,
                            in_=st,
                            func=mybir.ActivationFunctionType.Exp,
                            scale=scale,
                        )
                        nc.tensor.matmul(
                            ot_ps,
                            lhsT=v_t[kt][:, h, :],
                            rhs=ex,
                            start=(kt == 0),
                            stop=(kt == NKT - 1),
                        )
                    rec = rec_pool.tile([hd + 1, cs], F32R, name=f"rec{qg}_{h}",
                                        tag="rec")
                    nc.vector.reciprocal(
                        out=rec[hd : hd + 1, :], in_=ot_ps[hd : hd + 1, :]
                    )
                    bc = ps_bc.tile([hd, cs], F32, name=f"bc{qg}_{h}", tag="bc")
                    nc.tensor.matmul(
                        bc,
                        lhsT=ones_t[hd : hd + 1, :],
                        rhs=rec[hd : hd + 1, :],
                        start=True,
                        stop=True,
                    )
                    bc_sb = bcs_pool.tile([hd, cs], F32, name=f"bcs{qg}_{h}",
                                          tag="bcs")
                    nc.vector.tensor_copy(out=bc_sb, in_=bc)
                    nc.vector.tensor_mul(
                        out=ot_t[poff : poff + hd, h // HPT, :],
                        in0=ot_ps[0:hd, :],
                        in1=bc_sb,
                    )
                # out-projection for this q chunk (row-parallel Wo),
                # contracting a full head pair (K=128) per matmul
                for ob in range(OT):
                    ps = ps_mm.tile([P, cs], F32, name=f"po{qg}_{ob}", tag="psmm")
                    for hp in range(n_hp):
                        nc.tensor.matmul(
                            ps,
                            lhsT=wo_sb[hp][:, ob * P : (ob + 1) * P],
                            rhs=ot_t[:, hp, :],
                            start=(hp == 0),
                            stop=(hp == n_hp - 1),
                        )
                    osb = out_pool.tile([P, cs], F32, name=f"osb{qg}_{ob}",
                                        tag="osb")
                    nc.vector.tensor_copy(out=osb, in_=ps)
                    nc.sync.dma_start(
                        out=outT[ob * P : (ob + 1) * P, qsl], in_=osb
                    )
    nc.finalize()
    return nc


_NC_CACHE = {}


def _get_program(key, **kw):
    if key not in _NC_CACHE:
        _NC_CACHE[key] = build_core_program(**kw)
    return _NC_CACHE[key]


def kernel(xq, xk, xv, Wq, Wk, Wv, Wo, bo):
    m, nq, qkd = xq.shape
    nkv = xk.shape[1]
    vd = xv.shape[2]
    inner = Wq.shape[0]
    odim = Wo.shape[0]
    assert (m, nq, qkd, nkv, vd, inner, odim) == (4, 2048, 1024, 2048, 1024, 1024, 1024)
    n_cores = 8
    gheads = 2  # head-groups (cores per batch)
    gslice = inner // gheads  # 512 inner dims per head-group

    WqT = np.ascontiguousarray(Wq.T.astype(np.float32))
    WkT = np.ascontiguousarray(Wk.T.astype(np.float32))
    WvT = np.ascontiguousarray(Wv.T.astype(np.float32))
    WoT = np.ascontiguousarray(Wo.T.astype(np.float32))

    in_maps = []
    for c in range(n_cores):
        b, g = divmod(c, gheads)
        sl = slice(g * gslice, (g + 1) * gslice)
        in_maps.append(
            {
                "xqT": np.ascontiguousarray(np.asarray(xq[b], np.float32).T),
                "xkT": np.ascontiguousarray(np.asarray(xk[b], np.float32).T),
                "xvT": np.ascontiguousarray(np.asarray(xv[b], np.float32).T),
                "wqT": np.ascontiguousarray(WqT[:, sl]),
                "wkT": np.ascontiguousarray(WkT[:, sl]),
                "wvT": np.ascontiguousarray(WvT[:, sl]),
                "woT": np.ascontiguousarray(WoT[sl, :]),
            }
        )

    nc = _get_program("full")
    res = run_bass_kernel_spmd(nc, in_maps, core_ids=list(range(n_cores)))
    global _LAST_RESULTS
    _LAST_RESULTS = res
    out = np.empty((m, nq, odim), np.float32)
    for b in range(m):
        acc = res.results[gheads * b]["outT"].copy()
        for g in range(1, gheads):
            acc += res.results[gheads * b + g]["outT"]
        out[b] = acc.T + np.asarray(bo, np.float32)[None, :]
    return out


# revision 18
# speedup vs baseline: 1.2809x; 1.2809x over previous
"""Multi-head attention (BaseMultiHeadAttention) Trainium2 Bass kernel.

Problem: m=4, nq=nkv=2048, qk_dim=v_dim=1024, 16 heads x 64 head_dim,
fp32, out = softmax(Q K^T / 8) V projected by Wo + bo.

Sharding over 8 cores: core c = (batch b=c//2, head-group g=c%2).
Each core computes 8 heads of one batch:
  - Q^T, K^T projections in [d, n] layout (weights stationary; host
    supplies x^T so no on-device transposes are needed)
  - V in natural [n, d] layout, augmented with a ones column per head so
    the attention-weight matmul also produces the softmax denominator
  - S^T = K Q^T per (head, q-chunk of 512); exp via ScalarE (scale fused)
  - O^T = V^T-free accumulation: lhsT = V chunk, rhs = exp(S^T) chunk
  - normalize by broadcast-matmul of 1/denominator, then row-parallel
    out-projection with Wo^T; partial outputs summed on host (+ bias).

All matmuls run operands bitcast to float32r (tensor engine processes
fp32 data 12-bit-mantissa-rounded at full rate for free dims >= 256).
"""

import numpy as np

import concourse.bass as bass
from concourse import bacc
import concourse.mybir as mybir
import concourse.tile as tile
from concourse.bass_utils import run_bass_kernel_spmd

F32 = mybir.dt.float32
F32R = mybir.dt.float32r
P = 128


# --------------------------------------------------------------------------
# Workaround: the walrus build in this container rejects Drain instructions
# carrying more than one sync wait (CoreV3GenImpl setupSyncWait). Split the
# TileContext tail drain into a chain of drains, one wait each.
def _install_drain_patch():
    import concourse.tile as _tile
    import concourse.mybir as _mybir
    from concourse.vector_clock import ScopedClock as _ScopedClock

    if getattr(_tile.TileContext, "_drain_split_patch", False):
        return

    def _patched_drain_and_barrier(self, tick_clock, wait_clock):
        drain_inst = self.nc.sync.drain()
        wait_clock.add_sem_waits(
            drain_inst.ins, _ScopedClock({None: tick_clock.global_clock})
        )
        si = drain_inst.ins.sync_info
        if si is not None and len(si.on_wait) > 1:
            waits = list(si.on_wait)
            drain_inst.ins.sync_info = _mybir.SyncInfo(
                on_wait=[waits[0]], on_update=list(si.on_update)
            )
            for w in waits[1:]:
                extra = self.nc.sync.drain()
                extra.ins.sync_info = _mybir.SyncInfo(on_wait=[w], on_update=[])
        self.nc.all_engine_barrier()
        assert self.sems is not None
        popped = self.nc._tile_sem_poison_stack.pop()
        assert popped is self._sem_poison
        self.nc.clear_and_free_semaphores(list(self.sems.allocated().values()))
        self.nc.all_engine_barrier()

    _tile.TileContext._drain_and_barrier = _patched_drain_and_barrier
    _tile.TileContext._drain_split_patch = True


_install_drain_patch()


def build_core_program(
    nq=2048,
    nkv=2048,
    ckdim=1024,  # qk/v input feature dim
    n_heads=8,  # heads handled by this core
    hd=64,  # head dim
    odim=1024,  # output dim of Wo
    cs=512,  # free-dim chunk size for matmuls
    num_devices=8,
):
    """Emit the per-core SPMD program. Inputs (per core):
    xqT/xkT/xvT [ckdim, nq|nkv], wqT/wkT/wvT [ckdim, d_local],
    woT [d_local, odim]. Output: outT [odim, nq] (partial, pre-bias).
    """
    d_local = n_heads * hd
    scale = float(hd) ** -0.5
    KC = ckdim // P  # contraction chunks for projections
    DMT = d_local // P  # partition tiles of Q^T/K^T
    NQC = nq // cs  # q chunks
    NVC = nkv // cs  # kv chunks (for V projection)
    NKT = nkv // P  # kv partition tiles (attention contraction)
    OT = odim // P  # out-proj partition tiles
    SUB = cs // P  # 128-row subtiles per chunk
    HPT = P // hd  # heads per partition tile
    assert d_local % P == 0 and nq % cs == 0 and nkv % cs == 0
    assert odim % P == 0 and ckdim % P == 0 and cs % P == 0

    nc = bacc.Bacc(
        "TRN2", target_bir_lowering=False, debug=False, num_devices=num_devices
    )
    xqT = nc.declare_dram_parameter("xqT", [ckdim, nq], F32R, isOutput=False)
    xkT = nc.declare_dram_parameter("xkT", [ckdim, nkv], F32R, isOutput=False)
    xvT = nc.declare_dram_parameter("xvT", [ckdim, nkv], F32R, isOutput=False)
    wqT = nc.declare_dram_parameter("wqT", [ckdim, d_local], F32R, isOutput=False)
    wkT = nc.declare_dram_parameter("wkT", [ckdim, d_local], F32R, isOutput=False)
    wvT = nc.declare_dram_parameter("wvT", [ckdim, d_local], F32R, isOutput=False)
    woT = nc.declare_dram_parameter("woT", [d_local, odim], F32R, isOutput=False)
    outT = nc.declare_dram_parameter("outT", [odim, nq], F32, isOutput=True)

    with tile.TileContext(nc) as tc:
        import contextlib

        ctx = contextlib.ExitStack()
        with ctx:
            ctx.enter_context(
                nc.allow_low_precision(
                    reason="fp32r is fp32 with a 12-bit-rounded mantissa; "
                    "accumulation stays fp32 in PSUM"
                )
            )
            w_pool = ctx.enter_context(tc.tile_pool(name="w", bufs=1))
            wo_pool = ctx.enter_context(tc.tile_pool(name="wo", bufs=1))
            qt_pool = ctx.enter_context(tc.tile_pool(name="qt", bufs=1))
            kt_pool = ctx.enter_context(tc.tile_pool(name="kt", bufs=1))
            v_pool = ctx.enter_context(tc.tile_pool(name="v", bufs=1))
            xin_pool = ctx.enter_context(tc.tile_pool(name="xin", bufs=6))
            exp_pool = ctx.enter_context(tc.tile_pool(name="expp", bufs=4))
            ot_pool = ctx.enter_context(tc.tile_pool(name="ot", bufs=2))
            rec_pool = ctx.enter_context(tc.tile_pool(name="rec", bufs=2))
            rcp_pool = ctx.enter_context(tc.tile_pool(name="rcp", bufs=2))
            out_pool = ctx.enter_context(tc.tile_pool(name="outp", bufs=3))
            ones_pool = ctx.enter_context(tc.tile_pool(name="ones", bufs=1))
            ps_mm = ctx.enter_context(tc.tile_pool(name="ps_mm", bufs=4, space="PSUM"))
            ps_st = ctx.enter_context(tc.tile_pool(name="ps_st", bufs=2, space="PSUM"))

            # ---- resident weights --------------------------------------
            def load_w(dram, label):
                tiles = []
                for kc in range(KC):
                    t = w_pool.tile([P, d_local], F32R, name=f"{label}{kc}", tag=f"w{kc}")
                    nc.sync.dma_start(out=t, in_=dram[kc * P : (kc + 1) * P, :])
                    tiles.append(t)
                return tiles

            wq_sb = load_w(wqT, "wq")
            wk_sb = load_w(wkT, "wk")

            wo_sb = []
            for mt in range(DMT):
                t = wo_pool.tile([P, odim], F32R, name=f"wo{mt}")
                nc.sync.dma_start(out=t, in_=woT[mt * P : (mt + 1) * P, :])
                wo_sb.append(t)

            # ones row at partition hd*? -> used as lhsT for the
            # denominator broadcast matmul (base partition = hd)
            ones_t = ones_pool.tile([P, hd], F32R, name="ones_t")
            nc.vector.memset(ones_t.bitcast(F32), 1.0)

            # ---- projections -------------------------------------------
            # Q^T and K^T: [d_local, n] tiles; weights stationary per kc.
            def project_T(x_dram, w_sb, out_pool, label, n_tot):
                out_tiles = [
                    out_pool.tile([P, n_tot], F32R, name=f"{label}{mt}")
                    for mt in range(DMT)
                ]
                for nch in range(n_tot // cs):
                    xch = []
                    for kc in range(KC):
                        t = xin_pool.tile([P, cs], F32R, name=f"x_{label}_{nch}_{kc}",
                                          tag="xin")
                        nc.sync.dma_start(
                            out=t,
                            in_=x_dram[
                                kc * P : (kc + 1) * P, nch * cs : (nch + 1) * cs
                            ],
                        )
                        xch.append(t)
                    pss = [
                        ps_mm.tile([P, cs], F32, name=f"ps_{label}_{nch}_{mt}",
                                   tag="psmm")
                        for mt in range(DMT)
                    ]
                    for kc in range(KC):
                        for mt in range(DMT):
                            nc.tensor.matmul(
                                pss[mt],
                                lhsT=w_sb[kc][:, mt * P : (mt + 1) * P],
                                rhs=xch[kc],
                                start=(kc == 0),
                                stop=(kc == KC - 1),
                            )
                    for mt in range(DMT):
                        nc.vector.tensor_copy(
                            out=out_tiles[mt][:, nch * cs : (nch + 1) * cs],
                            in_=pss[mt],
                        )
                return out_tiles

            qt_sb = project_T(xqT, wq_sb, qt_pool, "qt", nq)
            kt_sb = project_T(xkT, wk_sb, kt_pool, "kt", nkv)

            # V natural layout, heads interleaved with a ones column:
            # v_t[nt] is [128, n_heads, hd+1]; [:, h, 0:hd] = V rows for
            # head h, [:, h, hd] = 1.0
            wv_sb = load_w(wvT, "wv")
            v_t = [
                v_pool.tile([P, n_heads, hd + 1], F32R, name=f"v{nt}")
                for nt in range(NKT)
            ]
            for nt in range(NKT):
                nc.vector.memset(v_t[nt][:, :, hd : hd + 1].bitcast(F32), 1.0)
            for nch in range(NVC):
                xch = []
                for kc in range(KC):
                    t = xin_pool.tile([P, cs], F32R, name=f"x_v_{nch}_{kc}", tag="xin")
                    nc.sync.dma_start(
                        out=t,
                        in_=xvT[kc * P : (kc + 1) * P, nch * cs : (nch + 1) * cs],
                    )
                    xch.append(t)
                pss = [
                    ps_mm.tile([P, d_local], F32, name=f"ps_v_{nch}_{sub}", tag="psmm")
                    for sub in range(SUB)
                ]
                for kc in range(KC):
                    for sub in range(SUB):
                        nc.tensor.matmul(
                            pss[sub],
                            lhsT=xch[kc][:, sub * P : (sub + 1) * P],
                            rhs=wv_sb[kc],
                            start=(kc == 0),
                            stop=(kc == KC - 1),
                        )
                for sub in range(SUB):
                    nt = nch * SUB + sub
                    nc.vector.tensor_copy(
                        out=v_t[nt][:, :, 0:hd],
                        in_=pss[sub].rearrange("p (h d) -> p h d", h=n_heads),
                    )

            # ---- attention + out-projection per q-chunk ----------------
            # ot_t stacks head pairs on partitions: [:, hp, :] holds heads
            # (2hp, 2hp+1) at partitions [0:hd) and [hd:2hd). The odd-head
            # normalize is a partition-shifted DVE write (64-channel ops may
            # write quadrants 2/3 from src 0/1).
            n_hp = n_heads // HPT
            assert HPT == 2 and NKT % 2 == 0
            for qg in range(NQC):
                qsl = slice(qg * cs, (qg + 1) * cs)
                ot_t = ot_pool.tile([P, n_hp, cs], F32R, name=f"ot{qg}", tag="ot")
                for hp in range(n_hp):
                    heads = (2 * hp, 2 * hp + 1)
                    ot_ps = {
                        h: ps_mm.tile([hd + 1, cs], F32, name=f"otps{qg}_{h}",
                                      tag="psmm")
                        for h in heads
                    }
                    # S^T pairs: both heads of the pair issue back-to-back
                    # into distinct PE row groups (partitions 0:64 / 64:128)
                    # and run concurrently. st/exp tiles span two kv tiles to
                    # halve the per-ACTIVATE overhead.
                    for kt2 in range(NKT // 2):
                        st = {
                            h: ps_st.tile([P, 2, cs], F32,
                                          name=f"st{qg}_{hp}_{kt2}_{h}", tag="st")
                            for h in heads
                        }
                        for jk in range(2):
                            kt = 2 * kt2 + jk
                            for h in heads:
                                poff = (h % HPT) * hd
                                nc.tensor.matmul(
                                    st[h][:, jk, :],
                                    lhsT=kt_sb[hp][
                                        poff : poff + hd, kt * P : (kt + 1) * P
                                    ],
                                    rhs=qt_sb[hp][poff : poff + hd, qsl],
                                    start=True,
                                    stop=True,
                                )
                        ex = {}
                        for h in heads:
                            ex[h] = exp_pool.tile([P, 2, cs], F32R,
                                                  name=f"ex{qg}_{kt2}_{h}",
                                                  tag="ex")
                            nc.scalar.activation(
                                out=ex[h],
                                in_=st[h],
                                func=mybir.ActivationFunctionType.Exp,
                                scale=scale,
                            )
                        for jk in range(2):
                            kt = 2 * kt2 + jk
                            for h in heads:
                                nc.tensor.matmul(
                                    ot_ps[h],
                                    lhsT=v_t[kt][:, h, :],
                                    rhs=ex[h][:, jk, :],
                                    start=(kt == 0),
                                    stop=(kt == NKT - 1),
                                )
                    for h in heads:
                        poff = (h % HPT) * hd
                        ot_sb = rec_pool.tile([hd + 1, cs], F32R,
                                              name=f"osb{qg}_{h}", tag="rec")
                        nc.vector.tensor_copy(out=ot_sb, in_=ot_ps[h])
                        rcp = rcp_pool.tile([hd + 1, cs], F32,
                                            name=f"rcp{qg}_{h}", tag="rcp")
                        nc.vector.reciprocal(
                            out=rcp[hd : hd + 1, :],
                            in_=ot_sb[hd : hd + 1, :].bitcast(F32),
                        )
                        # round the reciprocal into the fp32r row the
                        # broadcast matmul consumes
                        nc.vector.tensor_copy(
                            out=ot_sb[hd : hd + 1, :],
                            in_=rcp[hd : hd + 1, :],
                        )
                        bc = ps_mm.tile([hd, cs], F32, name=f"bc{qg}_{h}",
                                        tag="psmm")
                        nc.tensor.matmul(
                            bc,
                            lhsT=ones_t[hd : hd + 1, :],
                            rhs=ot_sb[hd : hd + 1, :],
                            start=True,
                            stop=True,
                        )
                        nc.vector.tensor_mul(
                            out=ot_t[poff : poff + hd, hp, :],
                            in0=ot_sb[0:hd, :].bitcast(F32),
                            in1=bc,
                        )
                # out-projection for this q chunk (row-parallel Wo),
                # contracting a full head pair (K=128) per matmul
                for ob in range(OT):
                    ps = ps_mm.tile([P, cs], F32, name=f"po{qg}_{ob}", tag="psmm")
                    for hp in range(n_hp):
                        nc.tensor.matmul(
                            ps,
                            lhsT=wo_sb[hp][:, ob * P : (ob + 1) * P],
                            rhs=ot_t[:, hp, :],
                            start=(hp == 0),
                            stop=(hp == n_hp - 1),
                        )
                    osb = out_pool.tile([P, cs], F32, name=f"osb{qg}_{ob}",
                                        tag="osb")
                    nc.vector.tensor_copy(out=osb, in_=ps)
                    nc.sync.dma_start(
                        out=outT[ob * P : (ob + 1) * P, qsl], in_=osb
                    )
    nc.finalize()
    return nc


_NC_CACHE = {}


def _get_program(key, **kw):
    if key not in _NC_CACHE:
        _NC_CACHE[key] = build_core_program(**kw)
    return _NC_CACHE[key]


def kernel(xq, xk, xv, Wq, Wk, Wv, Wo, bo):
    m, nq, qkd = xq.shape
    nkv = xk.shape[1]
    vd = xv.shape[2]
    inner = Wq.shape[0]
    odim = Wo.shape[0]
    assert (m, nq, qkd, nkv, vd, inner, odim) == (4, 2048, 1024, 2048, 1024, 1024, 1024)
    n_cores = 8
    gheads = 2  # head-groups (cores per batch)
    gslice = inner // gheads  # 512 inner dims per head-group

    WqT = np.ascontiguousarray(Wq.T.astype(np.float32))
    WkT = np.ascontiguousarray(Wk.T.astype(np.float32))
    WvT = np.ascontiguousarray(Wv.T.astype(np.float32))
    WoT = np.ascontiguousarray(Wo.T.astype(np.float32))

    in_maps = []
    for c in range(n_cores):
        b, g = divmod(c, gheads)
        sl = slice(g * gslice, (g + 1) * gslice)
        in_maps.append(
            {
                "xqT": np.ascontiguousarray(np.asarray(xq[b], np.float32).T),
                "xkT": np.ascontiguousarray(np.asarray(xk[b], np.float32).T),
                "xvT": np.ascontiguousarray(np.asarray(xv[b], np.float32).T),
                "wqT": np.ascontiguousarray(WqT[:, sl]),
                "wkT": np.ascontiguousarray(WkT[:, sl]),
                "wvT": np.ascontiguousarray(WvT[:, sl]),
                "woT": np.ascontiguousarray(WoT[sl, :]),
            }
        )

    nc = _get_program("full")
    res = run_bass_kernel_spmd(nc, in_maps, core_ids=list(range(n_cores)))
    global _LAST_RESULTS
    _LAST_RESULTS = res
    out = np.empty((m, nq, odim), np.float32)
    for b in range(m):
        acc = res.results[gheads * b]["outT"].copy()
        for g in range(1, gheads):
            acc += res.results[gheads * b + g]["outT"]
        out[b] = acc.T + np.asarray(bo, np.float32)[None, :]
    return out


# revision 19
# speedup vs baseline: 1.7176x; 1.3409x over previous
"""Multi-head attention (BaseMultiHeadAttention) Trainium2 Bass kernel.

Problem: m=4, nq=nkv=2048, qk_dim=v_dim=1024, 16 heads x 64 head_dim,
fp32, out = softmax(Q K^T / 8) V projected by Wo + bo.

Sharding over 8 cores: core c = (batch b=c//2, head-group g=c%2).
Each core computes 8 heads of one batch:
  - Q^T, K^T projections in [d, n] layout (weights stationary; host
    supplies x^T so no on-device transposes are needed)
  - V in natural [n, d] layout, augmented with a ones column per head so
    the attention-weight matmul also produces the softmax denominator
  - S^T = K Q^T per (head, q-chunk of 512); exp via ScalarE (scale fused)
  - O^T = V^T-free accumulation: lhsT = V chunk, rhs = exp(S^T) chunk
  - normalize by broadcast-matmul of 1/denominator, then row-parallel
    out-projection with Wo^T; partial outputs summed on host (+ bias).

All matmuls run operands bitcast to float32r (tensor engine processes
fp32 data 12-bit-mantissa-rounded at full rate for free dims >= 256).
"""

import numpy as np

import concourse.bass as bass
from concourse import bacc
import concourse.mybir as mybir
import concourse.tile as tile
from concourse.bass_utils import run_bass_kernel_spmd

F32 = mybir.dt.float32
F32R = mybir.dt.float32r
P = 128


# --------------------------------------------------------------------------
# Workaround: the walrus build in this container rejects Drain instructions
# carrying more than one sync wait (CoreV3GenImpl setupSyncWait). Split the
# TileContext tail drain into a chain of drains, one wait each.
def _install_drain_patch():
    import concourse.tile as _tile
    import concourse.mybir as _mybir
    from concourse.vector_clock import ScopedClock as _ScopedClock

    if getattr(_tile.TileContext, "_drain_split_patch", False):
        return

    def _patched_drain_and_barrier(self, tick_clock, wait_clock):
        drain_inst = self.nc.sync.drain()
        wait_clock.add_sem_waits(
            drain_inst.ins, _ScopedClock({None: tick_clock.global_clock})
        )
        si = drain_inst.ins.sync_info
        if si is not None and len(si.on_wait) > 1:
            waits = list(si.on_wait)
            drain_inst.ins.sync_info = _mybir.SyncInfo(
                on_wait=[waits[0]], on_update=list(si.on_update)
            )
            for w in waits[1:]:
                extra = self.nc.sync.drain()
                extra.ins.sync_info = _mybir.SyncInfo(on_wait=[w], on_update=[])
        self.nc.all_engine_barrier()
        assert self.sems is not None
        popped = self.nc._tile_sem_poison_stack.pop()
        assert popped is self._sem_poison
        self.nc.clear_and_free_semaphores(list(self.sems.allocated().values()))
        self.nc.all_engine_barrier()

    _tile.TileContext._drain_and_barrier = _patched_drain_and_barrier
    _tile.TileContext._drain_split_patch = True


_install_drain_patch()


def build_core_program(
    nq=2048,
    nkv=2048,
    ckdim=1024,  # qk/v input feature dim
    n_heads=8,  # heads handled by this core
    hd=64,  # head dim
    odim=1024,  # output dim of Wo
    cs=512,  # free-dim chunk size for matmuls
    num_devices=8,
):
    """Emit the per-core SPMD program. Inputs (per core):
    xqT/xkT/xvT [ckdim, nq|nkv], wqT/wkT/wvT [ckdim, d_local],
    woT [d_local, odim]. Output: outT [odim, nq] (partial, pre-bias).
    """
    d_local = n_heads * hd
    scale = float(hd) ** -0.5
    KC = ckdim // P  # contraction chunks for projections
    DMT = d_local // P  # partition tiles of Q^T/K^T
    NQC = nq // cs  # q chunks
    NVC = nkv // cs  # kv chunks (for V projection)
    NKT = nkv // P  # kv partition tiles (attention contraction)
    OT = odim // P  # out-proj partition tiles
    SUB = cs // P  # 128-row subtiles per chunk
    HPT = P // hd  # heads per partition tile
    assert d_local % P == 0 and nq % cs == 0 and nkv % cs == 0
    assert odim % P == 0 and ckdim % P == 0 and cs % P == 0

    nc = bacc.Bacc(
        "TRN2", target_bir_lowering=False, debug=False, num_devices=num_devices
    )
    xqT = nc.declare_dram_parameter("xqT", [ckdim, nq], F32R, isOutput=False)
    xkT = nc.declare_dram_parameter("xkT", [ckdim, nkv], F32R, isOutput=False)
    xvT = nc.declare_dram_parameter("xvT", [ckdim, nkv], F32R, isOutput=False)
    wqT = nc.declare_dram_parameter("wqT", [ckdim, d_local], F32R, isOutput=False)
    wkT = nc.declare_dram_parameter("wkT", [ckdim, d_local], F32R, isOutput=False)
    wvT = nc.declare_dram_parameter("wvT", [ckdim, d_local], F32R, isOutput=False)
    woT = nc.declare_dram_parameter("woT", [d_local, odim], F32R, isOutput=False)
    outT = nc.declare_dram_parameter("outT", [odim, nq], F32, isOutput=True)

    with tile.TileContext(nc) as tc:
        import contextlib

        ctx = contextlib.ExitStack()
        with ctx:
            ctx.enter_context(
                nc.allow_low_precision(
                    reason="fp32r is fp32 with a 12-bit-rounded mantissa; "
                    "accumulation stays fp32 in PSUM"
                )
            )
            w_pool = ctx.enter_context(tc.tile_pool(name="w", bufs=1))
            wo_pool = ctx.enter_context(tc.tile_pool(name="wo", bufs=1))
            qt_pool = ctx.enter_context(tc.tile_pool(name="qt", bufs=1))
            kt_pool = ctx.enter_context(tc.tile_pool(name="kt", bufs=1))
            v_pool = ctx.enter_context(tc.tile_pool(name="v", bufs=1))
            xin_pool = ctx.enter_context(tc.tile_pool(name="xin", bufs=8))
            exp_pool = ctx.enter_context(tc.tile_pool(name="expp", bufs=4))
            ot_pool = ctx.enter_context(tc.tile_pool(name="ot", bufs=2))
            rec_pool = ctx.enter_context(tc.tile_pool(name="rec", bufs=2))
            rcp_pool = ctx.enter_context(tc.tile_pool(name="rcp", bufs=4))
            out_pool = ctx.enter_context(tc.tile_pool(name="outp", bufs=2))
            ones_pool = ctx.enter_context(tc.tile_pool(name="ones", bufs=1))
            ps_mm = ctx.enter_context(tc.tile_pool(name="ps_mm", bufs=4, space="PSUM"))
            ps_st = ctx.enter_context(tc.tile_pool(name="ps_st", bufs=2, space="PSUM"))

            # ---- resident weights --------------------------------------
            def load_w(dram, label):
                tiles = []
                for kc in range(KC):
                    t = w_pool.tile([P, d_local], F32R, name=f"{label}{kc}", tag=f"w{kc}")
                    nc.sync.dma_start(out=t, in_=dram[kc * P : (kc + 1) * P, :])
                    tiles.append(t)
                return tiles

            wq_sb = load_w(wqT, "wq")
            wk_sb = load_w(wkT, "wk")

            wo_sb = []
            for mt in range(DMT):
                t = wo_pool.tile([P, odim], F32R, name=f"wo{mt}")
                nc.sync.dma_start(out=t, in_=woT[mt * P : (mt + 1) * P, :])
                wo_sb.append(t)

            # ones row at partition hd*? -> used as lhsT for the
            # denominator broadcast matmul (base partition = hd)
            ones_t = ones_pool.tile([P, hd], F32R, name="ones_t")
            nc.vector.memset(ones_t.bitcast(F32), 1.0)

            # ---- projections -------------------------------------------
            # Q^T and K^T: [d_local, n] tiles; weights stationary per kc.
            def project_T(x_dram, w_sb, out_pool, label, n_tot):
                out_tiles = [
                    out_pool.tile([P, n_tot], F32R, name=f"{label}{mt}")
                    for mt in range(DMT)
                ]
                for nch in range(n_tot // cs):
                    xch = []
                    for kc in range(KC):
                        t = xin_pool.tile([P, cs], F32R, name=f"x_{label}_{nch}_{kc}",
                                          tag="xin")
                        nc.sync.dma_start(
                            out=t,
                            in_=x_dram[
                                kc * P : (kc + 1) * P, nch * cs : (nch + 1) * cs
                            ],
                        )
                        xch.append(t)
                    pss = [
                        ps_mm.tile([P, cs], F32, name=f"ps_{label}_{nch}_{mt}",
                                   tag="psmm")
                        for mt in range(DMT)
                    ]
                    for kc in range(KC):
                        for mt in range(DMT):
                            nc.tensor.matmul(
                                pss[mt],
                                lhsT=w_sb[kc][:, mt * P : (mt + 1) * P],
                                rhs=xch[kc],
                                start=(kc == 0),
                                stop=(kc == KC - 1),
                            )
                    for mt in range(DMT):
                        nc.vector.tensor_copy(
                            out=out_tiles[mt][:, nch * cs : (nch + 1) * cs],
                            in_=pss[mt],
                        )
                return out_tiles

            qt_sb = project_T(xqT, wq_sb, qt_pool, "qt", nq)
            kt_sb = project_T(xkT, wk_sb, kt_pool, "kt", nkv)

            # V natural layout, heads interleaved with a ones column:
            # v_t[nt] is [128, n_heads, hd+1]; [:, h, 0:hd] = V rows for
            # head h, [:, h, hd] = 1.0
            wv_sb = load_w(wvT, "wv")
            v_t = [
                v_pool.tile([P, n_heads, hd + 1], F32R, name=f"v{nt}")
                for nt in range(NKT)
            ]
            for nt in range(NKT):
                nc.vector.memset(v_t[nt][:, :, hd : hd + 1].bitcast(F32), 1.0)
            for nch in range(NVC):
                xch = []
                for kc in range(KC):
                    t = xin_pool.tile([P, cs], F32R, name=f"x_v_{nch}_{kc}", tag="xin")
                    nc.sync.dma_start(
                        out=t,
                        in_=xvT[kc * P : (kc + 1) * P, nch * cs : (nch + 1) * cs],
                    )
                    xch.append(t)
                pss = [
                    ps_mm.tile([P, d_local], F32, name=f"ps_v_{nch}_{sub}", tag="psmm")
                    for sub in range(SUB)
                ]
                for kc in range(KC):
                    for sub in range(SUB):
                        nc.tensor.matmul(
                            pss[sub],
                            lhsT=xch[kc][:, sub * P : (sub + 1) * P],
                            rhs=wv_sb[kc],
                            start=(kc == 0),
                            stop=(kc == KC - 1),
                        )
                for sub in range(SUB):
                    nt = nch * SUB + sub
                    nc.vector.tensor_copy(
                        out=v_t[nt][:, :, 0:hd],
                        in_=pss[sub].rearrange("p (h d) -> p h d", h=n_heads),
                    )

            # ---- attention + out-projection per q-chunk ----------------
            # ot_t stacks head pairs on partitions: [:, hp, :] holds heads
            # (2hp, 2hp+1) at partitions [0:hd) and [hd:2hd). The odd-head
            # normalize is a partition-shifted DVE write (64-channel ops may
            # write quadrants 2/3 from src 0/1).
            n_hp = n_heads // HPT
            assert HPT == 2 and NKT % 2 == 0
            for qg in range(NQC):
                qsl = slice(qg * cs, (qg + 1) * cs)
                ot_t = ot_pool.tile([P, n_hp, cs], F32R, name=f"ot{qg}", tag="ot")
                for hp in range(n_hp):
                    heads = (2 * hp, 2 * hp + 1)
                    ot_ps = {
                        h: ps_mm.tile([hd + 1, cs], F32, name=f"otps{qg}_{h}",
                                      tag="psmm")
                        for h in heads
                    }
                    # S^T pairs: both heads of the pair issue back-to-back
                    # into distinct PE row groups (partitions 0:64 / 64:128)
                    # and run concurrently. st/exp tiles span two kv tiles to
                    # halve the per-ACTIVATE overhead.
                    for kt2 in range(NKT // 2):
                        st = {
                            h: ps_st.tile([P, 2, cs], F32,
                                          name=f"st{qg}_{hp}_{kt2}_{h}", tag="st")
                            for h in heads
                        }
                        for jk in range(2):
                            kt = 2 * kt2 + jk
                            for h in heads:
                                poff = (h % HPT) * hd
                                nc.tensor.matmul(
                                    st[h][:, jk, :],
                                    lhsT=kt_sb[hp][
                                        poff : poff + hd, kt * P : (kt + 1) * P
                                    ],
                                    rhs=qt_sb[hp][poff : poff + hd, qsl],
                                    start=True,
                                    stop=True,
                                )
                        ex = {}
                        for h in heads:
                            ex[h] = exp_pool.tile([P, 2, cs], F32R,
                                                  name=f"ex{qg}_{kt2}_{h}",
                                                  tag="ex")
                            nc.scalar.activation(
                                out=ex[h],
                                in_=st[h],
                                func=mybir.ActivationFunctionType.Exp,
                                scale=scale,
                            )
                        for jk in range(2):
                            kt = 2 * kt2 + jk
                            for h in heads:
                                nc.tensor.matmul(
                                    ot_ps[h],
                                    lhsT=v_t[kt][:, h, :],
                                    rhs=ex[h][:, jk, :],
                                    start=(kt == 0),
                                    stop=(kt == NKT - 1),
                                )
                    for h in heads:
                        poff = (h % HPT) * hd
                        ot_sb = rec_pool.tile([hd + 1, cs], F32R,
                                              name=f"osb{qg}_{h}", tag="rec")
                        nc.vector.tensor_copy(out=ot_sb, in_=ot_ps[h])
                        # broadcast the RAW denominator row (no reciprocal on
                        # the PE-feeding path), then invert the broadcast on
                        # DVE at partition 0 where the approx op is valid
                        bc = ps_mm.tile([hd, cs], F32, name=f"bc{qg}_{h}",
                                        tag="psmm")
                        nc.tensor.matmul(
                            bc,
                            lhsT=ones_t[hd : hd + 1, :],
                            rhs=ot_sb[hd : hd + 1, :],
                            start=True,
                            stop=True,
                        )
                        bcr = rcp_pool.tile([hd, cs], F32,
                                            name=f"bcr{qg}_{h}", tag="rcp")
                        nc.vector.tensor_copy(out=bcr, in_=bc)
                        bci = rcp_pool.tile([hd, cs], F32,
                                            name=f"bci{qg}_{h}", tag="rcp")
                        nc.vector.reciprocal_approx_fast(out=bci, in_=bcr)
                        nc.vector.tensor_mul(
                            out=ot_t[poff : poff + hd, hp, :],
                            in0=ot_sb[0:hd, :].bitcast(F32),
                            in1=bci,
                        )
                # out-projection for this q chunk (row-parallel Wo),
                # contracting a full head pair (K=128) per matmul
                for ob in range(OT):
                    ps = ps_mm.tile([P, cs], F32, name=f"po{qg}_{ob}", tag="psmm")
                    for hp in range(n_hp):
                        nc.tensor.matmul(
                            ps,
                            lhsT=wo_sb[hp][:, ob * P : (ob + 1) * P],
                            rhs=ot_t[:, hp, :],
                            start=(hp == 0),
                            stop=(hp == n_hp - 1),
                        )
                    osb = out_pool.tile([P, cs], F32, name=f"osb{qg}_{ob}",
                                        tag="osb")
                    nc.vector.tensor_copy(out=osb, in_=ps)
                    nc.sync.dma_start(
                        out=outT[ob * P : (ob + 1) * P, qsl], in_=osb
                    )
    nc.finalize()
    return nc


_NC_CACHE = {}


def _get_program(key, **kw):
    if key not in _NC_CACHE:
        _NC_CACHE[key] = build_core_program(**kw)
    return _NC_CACHE[key]


def kernel(xq, xk, xv, Wq, Wk, Wv, Wo, bo):
    m, nq, qkd = xq.shape
    nkv = xk.shape[1]
    vd = xv.shape[2]
    inner = Wq.shape[0]
    odim = Wo.shape[0]
    assert (m, nq, qkd, nkv, vd, inner, odim) == (4, 2048, 1024, 2048, 1024, 1024, 1024)
    n_cores = 8
    gheads = 2  # head-groups (cores per batch)
    gslice = inner // gheads  # 512 inner dims per head-group

    WqT = np.ascontiguousarray(Wq.T.astype(np.float32))
    WkT = np.ascontiguousarray(Wk.T.astype(np.float32))
    WvT = np.ascontiguousarray(Wv.T.astype(np.float32))
    WoT = np.ascontiguousarray(Wo.T.astype(np.float32))

    in_maps = []
    for c in range(n_cores):
        b, g = divmod(c, gheads)
        sl = slice(g * gslice, (g + 1) * gslice)
        in_maps.append(
            {
                "xqT": np.ascontiguousarray(np.asarray(xq[b], np.float32).T),
                "xkT": np.ascontiguousarray(np.asarray(xk[b], np.float32).T),
                "xvT": np.ascontiguousarray(np.asarray(xv[b], np.float32).T),
                "wqT": np.ascontiguousarray(WqT[:, sl]),
                "wkT": np.ascontiguousarray(WkT[:, sl]),
                "wvT": np.ascontiguousarray(WvT[:, sl]),
                "woT": np.ascontiguousarray(WoT[sl, :]),
            }
        )

    nc = _get_program("full")
    res = run_bass_kernel_spmd(nc, in_maps, core_ids=list(range(n_cores)))
    global _LAST_RESULTS
    _LAST_RESULTS = res
    out = np.empty((m, nq, odim), np.float32)
    for b in range(m):
        acc = res.results[gheads * b]["outT"].copy()
        for g in range(1, gheads):
            acc += res.results[gheads * b + g]["outT"]
        out[b] = acc.T + np.asarray(bo, np.float32)[None, :]
    return out
